# revision 11
# baseline (speedup 1.0000x reference)
"""Doc self-attention kernel for Trainium2 (Bass/Tile), 8-core data-parallel.

Reference computation (per batch b):
    P   = D_b @ W^T            [N, H]
    L   = P @ D_b^T            [N, N]
    A   = softmax(L, axis=-1)
    out = A @ D_b              [N, DIN]

Sharding: B=8 batches -> one batch per NeuronCore (pure data parallel, no
collectives). Per core everything stays SBUF-resident:
  - Dt  = D_b^T  [DIN, N]   (host-pretransposed)   -> lhsT/rhs for P and L
  - Dn  = D_b    [N, DIN]                           -> rhs for A@D
  - Wt  = W^T    [DIN, H]   (host-pretransposed)   -> lhsT for P
Matmuls run in float32r (PE full-rate fp32 streaming); fp32r operands must be
produced by a rounding op, so DMA loads stage through fp32 tiles and round on
DVE/ACT, and PSUM->SBUF copies round on the way out.

Per 128-row block: scores land in PSUM 512 cols at a time, row-max is reduced
per chunk as it completes, exp(+row-sum) is fused on the scalar engine, E
blocks are PE-transposed into the lhsT for the A@D accumulation, and 1/rowsum
is folded into the final PSUM->SBUF copy. Blocks are software-pipelined: the
A@D work of block i-1 fills the PE while block i's softmax stats are computed.
"""

import numpy as np

import concourse.bass as bass
import concourse.tile as tile
from concourse import mybir
from concourse.bass_utils import run_bass_kernel_spmd
from concourse.masks import make_identity

B, N, DIN, DHID = 8, 2048, 768, 768
P = 128            # partitions
NB = N // P        # 16 row blocks
KB = DIN // P      # 6 contraction chunks
HB = DHID // P     # 6 hidden chunks
MC = 512           # score-matrix column chunk (one PSUM bank, fp32)
NMC = N // MC      # 4

F32 = mybir.dt.float32
F32R = mybir.dt.float32r

USE_F32R = True    # float32r streams fp32 through the PE at 1 cycle/row
REPEAT = 1         # repeat the body (timing harness differencing)
MM_DT = F32R if USE_F32R else F32


class SplitDrainTileContext(tile.TileContext):
    """This walrus build allows at most one sem wait per instruction, but the
    Tile scheduler freely attaches several (and the stock kernel-tail drain
    carries one wait per outstanding engine/queue). Split every extra wait
    onto a standalone same-engine NoOp placed immediately before the
    instruction; sequencers execute their stream in order, so semantics are
    unchanged."""

    def _split_multi_waits(self):
        nc = self.nc
        for bb in nc.main_func.blocks:
            need = any(
                ins.sync_info and ins.sync_info.on_wait
                and len(ins.sync_info.on_wait) > 1
                for ins in bb.instructions
            )
            if not need:
                continue
            new_insts = []
            for ins in bb.instructions:
                si = ins.sync_info
                waits = list(si.on_wait) if (si and si.on_wait) else []
                if len(waits) > 1:
                    for w in waits[:-1]:
                        nop = mybir.InstNoOp(
                            name=nc.get_next_instruction_name(),
                            engine=ins.engine,
                            ins=[], outs=[],
                            sync_info=mybir.SyncInfo(on_wait=[w], on_update=[]),
                            bass_nofuse=True,
                        )
                        new_insts.append(nop)
                    si.on_wait = waits[-1:]
                new_insts.append(ins)
            bb.instructions = new_insts

    def _drain_and_barrier(self, tick_clock, wait_clock):
        from concourse.tile import ScopedClock

        self._split_multi_waits()
        nop = self.nc.sync.nop(nofuse=True)
        wait_clock.add_sem_waits(
            nop.ins, ScopedClock({None: tick_clock.global_clock})
        )
        si = nop.ins.sync_info
        waits = list(si.on_wait or []) if si else []
        if len(waits) > 1:
            si.on_wait = waits[:1]
            for g in range(1, len(waits)):
                n2 = self.nc.sync.nop(nofuse=True)
                n2.ins.sync_info = mybir.SyncInfo(
                    on_wait=[waits[g]], on_update=[]
                )
        self.nc.sync.drain()
        self.nc.all_engine_barrier()
        assert self.sems is not None
        popped = self.nc._tile_sem_poison_stack.pop()
        assert popped is self._sem_poison
        self.nc.clear_and_free_semaphores(list(self.sems.allocated().values()))
        self.nc.all_engine_barrier()


def build_program():
    nc = bass.Bass()
    Dn_d = nc.declare_dram_parameter("Dn", [N, DIN], F32, isOutput=False)
    Dt_d = nc.declare_dram_parameter("Dt", [DIN, N], F32, isOutput=False)
    Wt_d = nc.declare_dram_parameter("Wt", [DIN, DHID], F32, isOutput=False)
    OUT_d = nc.declare_dram_parameter("OUT", [N, DIN], F32, isOutput=True)

    with SplitDrainTileContext(nc) as tc:
        with (
            tc.tile_pool(name="resident", bufs=1) as resident,
            tc.tile_pool(name="stage", bufs=2) as stage,
            tc.tile_pool(name="stats", bufs=2) as stats,
            tc.tile_pool(name="e_pool", bufs=2) as e_pool,
            tc.tile_pool(name="et_pool", bufs=2) as et_pool,
            tc.tile_pool(name="o_pool", bufs=2) as o_pool,
        ):
            for rep in range(REPEAT):
                identity = resident.tile([P, P], F32, tag="identity")
                make_identity(nc, identity)

                # Load fp32 into staging, round into fp32r residents; the
                # rounding copies alternate DVE/ACT so they run in parallel.
                rounders = [nc.vector.tensor_copy, nc.scalar.copy]

                def load_rounded(pool_tag, shape, dram_slice, ridx):
                    t = resident.tile(shape, MM_DT, tag=pool_tag)
                    if USE_F32R:
                        stg = stage.tile(shape, F32, tag=f"stg{shape[1]}")
                        nc.sync.dma_start(out=stg, in_=dram_slice)
                        rounders[ridx % 2](out=t, in_=stg)
                    else:
                        nc.sync.dma_start(out=t, in_=dram_slice)
                    return t

                dt_tiles = []
                for k in range(KB):
                    t = load_rounded(f"dt{k}", [P, N],
                                     Dt_d[k * P:(k + 1) * P, :], k)
                    dt_tiles.append(t)
                wt_tiles = []
                for k in range(KB):
                    t = load_rounded(f"wt{k}", [P, DHID],
                                     Wt_d[k * P:(k + 1) * P, :], k)
                    wt_tiles.append(t)
                dn_tiles = []
                for j in range(NB):
                    t = load_rounded(f"dn{j}", [P, DIN],
                                     Dn_d[j * P:(j + 1) * P, :], j)
                    dn_tiles.append(t)
                pt_tiles = []
                for h in range(HB):
                    t = resident.tile([P, N], MM_DT, tag=f"pt{h}")
                    pt_tiles.append(t)

                # Phase 1: Pt[h, n] = sum_d W[h, d] * Dt[d, n]
                with tc.tile_pool(name=f"psum_p{rep}", bufs=2,
                                  space="PSUM") as pp:
                    for h in range(HB):
                        for c in range(NMC):
                            ps = pp.tile([P, MC], F32, tag="p")
                            for d in range(KB):
                                nc.tensor.matmul(
                                    ps,
                                    lhsT=wt_tiles[d][:, h * P:(h + 1) * P],
                                    rhs=dt_tiles[d][:, c * MC:(c + 1) * MC],
                                    start=(d == 0),
                                    stop=(d == KB - 1),
                                )
                            # PSUM->SBUF copy rounds to fp32r on the way out
                            nc.scalar.copy(
                                out=pt_tiles[h][:, c * MC:(c + 1) * MC],
                                in_=ps)

                # Phase 2, software-pipelined across row blocks
                with (
                    tc.tile_pool(name=f"psum_L{rep}", bufs=4,
                                 space="PSUM") as pl,
                    tc.tile_pool(name=f"psum_t{rep}", bufs=2,
                                 space="PSUM") as ptp,
                    tc.tile_pool(name=f"psum_o{rep}", bufs=1,
                                 space="PSUM") as po,
                ):
                    def softmax_block(i):
                        """Scores + stabilized exp for row block i."""
                        l_chunks = []
                        pmax = stats.tile([P, NMC], F32, tag="pmax")
                        for c in range(NMC):
                            lp = pl.tile([P, MC], F32, tag="L")
                            for h in range(HB):
                                nc.tensor.matmul(
                                    lp,
                                    lhsT=pt_tiles[h][:, i * P:(i + 1) * P],
                                    rhs=dt_tiles[h][:, c * MC:(c + 1) * MC],
                                    start=(h == 0),
                                    stop=(h == HB - 1),
                                )
                            # negated per-chunk row max (bias for exp)
                            nc.vector.tensor_reduce(
                                out=pmax[:, c:c + 1], in_=lp,
                                axis=mybir.AxisListType.X,
                                op=mybir.AluOpType.max,
                                negate=True,
                            )
                            l_chunks.append(lp)
                        negmax = stats.tile([P, 1], F32, tag="negmax")
                        nc.vector.tensor_reduce(
                            out=negmax, in_=pmax,
                            axis=mybir.AxisListType.X, op=mybir.AluOpType.min,
                        )
                        psums = stats.tile([P, NMC], F32, tag="psums")
                        e_tile = e_pool.tile([P, N], F32, tag="e")
                        for c in range(NMC):
                            nc.scalar.activation(
                                out=e_tile[:, c * MC:(c + 1) * MC],
                                in_=l_chunks[c],
                                func=mybir.ActivationFunctionType.Exp,
                                bias=negmax, scale=1.0,
                                accum_out=psums[:, c:c + 1],
                            )
                        rowsum = stats.tile([P, 1], F32, tag="rowsum")
                        nc.vector.tensor_reduce(
                            out=rowsum, in_=psums,
                            axis=mybir.AxisListType.X, op=mybir.AluOpType.add,
                        )
                        rinv = stats.tile([P, 1], F32, tag="rinv")
                        nc.vector.reciprocal(out=rinv, in_=rowsum)
                        return e_tile, rinv

                    def av_block(i, e_tile, rinv):
                        """A@D for row block i from its unnormalized E."""
                        op_ = po.tile([P, DIN], F32, tag="o")
                        for j in range(NB):
                            tp = ptp.tile([P, P], F32, tag="t")
                            nc.tensor.transpose(
                                tp, e_tile[:, j * P:(j + 1) * P], identity)
                            et = et_pool.tile([P, P], MM_DT, tag="et")
                            nc.vector.tensor_copy(out=et, in_=tp)
                            nc.tensor.matmul(
                                op_[:, 0:512], lhsT=et,
                                rhs=dn_tiles[j][:, 0:512],
                                start=(j == 0), stop=(j == NB - 1),
                            )
                            nc.tensor.matmul(
                                op_[:, 512:768], lhsT=et,
                                rhs=dn_tiles[j][:, 512:768],
                                start=(j == 0), stop=(j == NB - 1),
                            )
                        o_sb = o_pool.tile([P, DIN], F32, tag="osb")
                        nc.scalar.mul(out=o_sb, in_=op_, mul=rinv)
                        nc.sync.dma_start(
                            out=OUT_d[i * P:(i + 1) * P, :], in_=o_sb)

                    prev = None
                    for i in range(NB):
                        cur = softmax_block(i)
                        if prev is not None:
                            av_block(*prev)
                        prev = (i, *cur)
                    av_block(*prev)
    return nc


_cached_nc = None


def _get_program():
    global _cached_nc
    if _cached_nc is None:
        _cached_nc = build_program()
    return _cached_nc


def _make_in_maps(D, W):
    Wt = np.ascontiguousarray(W.T)
    in_maps = []
    for b in range(B):
        Db = np.ascontiguousarray(D[b])
        in_maps.append({
            "Dn": Db,
            "Dt": np.ascontiguousarray(Db.T),
            "Wt": Wt,
        })
    return in_maps


def kernel(D, W):
    D = np.ascontiguousarray(np.asarray(D, dtype=np.float32))
    W = np.ascontiguousarray(np.asarray(W, dtype=np.float32))
    nc = _get_program()
    res = run_bass_kernel_spmd(nc, _make_in_maps(D, W), list(range(B)))
    return np.stack([res.results[b]["OUT"] for b in range(B)], axis=0)
